# revision 2
# baseline (speedup 1.0000x reference)
"""Quantized 3x3 conv (8-bit symmetric STE quantization of x and w, then
stride-1 pad-1 conv) on 8 Trainium2 NeuronCores.

Strategy (v2)
-------------
Data-parallel over batch: 4 images per core (32/8).  Host pre-quantizes
both operands to integer grids (exactly the reference fp32 math):
  * x -> kx int8 in [-127,127]  (1/4 the DMA bytes of fp32)
  * w -> kw bf16 lhsT [ci, tap, co], duplicated into both partition halves
Per core:
  * kx int8 DMAs in chunked; a single ACT/DVE pass upconverts to bf16
    while relaying out to a 58-wide zero-padded grid (pad = conv padding).
  * conv = 9 shifted matmuls (K=ci=64, M=co=128) accumulating in PSUM.
    Integer products accumulate exactly in fp32 PSUM (|sum| <= 9.3e6 < 2^24).
    Two images run concurrently on the PE via row-tiling: image (2g) on
    partitions 0-63, image (2g+1) on partitions 64-127.
  * PSUM -> SBUF copy applies the final scale s2 = step_x*step_w, emits
    fp16 (rel err ~5e-4, half the output DMA bytes), strips pad columns.
  * Host upcasts fp16 -> fp32.
"""

import os

import numpy as np
import ml_dtypes

import concourse.bass as bass
import concourse.mybir as mybir
import concourse.tile as tile
from concourse import bacc
from concourse.bass_utils import run_bass_kernel_spmd

dt = mybir.dt

N_CORES = 8
NPC = 4                # images per core
CI, CO = 64, 128
H = W = 56
WP = 58                # padded row width (56 + 2)
LEAD = 4               # guard elems before the padded grid
IMG_ELEMS = LEAD + WP * WP + 8   # 4 + 3364 + 8 = 3376
PACK = H * W           # 3136
H0S = [1 + 8 * i for i in range(7)]   # padded-row start of each 8-row block
BLK = 8 * WP           # 464 psum columns per block
N_WARM = 24            # PE warmup matmuls (HAM un-throttle)

_PROG_CACHE = {}


def _build_program(s2):
    """One SPMD program; per-core shards differ only through in_maps.

    s2 (=step_x*step_w) is embedded as an immediate — the program is
    specialized per (alpha_x, alpha_w) value and cached.  Immediates keep
    every instruction at <=1 semaphore wait (the TRN2 TensorScalar ISA
    slot limit walrus enforces)."""
    s2 = float(np.float32(s2))
    nc = bacc.Bacc(None)
    x_in = nc.declare_dram_parameter("x", [NPC * CI, PACK], dt.int8, isOutput=False)
    wq_in = nc.declare_dram_parameter("wq", [128, 9, CO], dt.bfloat16, isOutput=False)
    out = nc.declare_dram_parameter("out", [NPC * CO, PACK], dt.float16, isOutput=True)

    # input chunks (data-row ranges) and the block groups they unlock.
    # The first two chunks are tiny so block 0's matmuls start as early as
    # possible; trailing single-block groups shrink the output-DMA tail.
    CHUNKS = [(0, 5), (5, 9), (9, 25), (25, 41), (41, 56)]
    ITERS = [[0], [1, 2], [3, 4], [5], [6]]

    with tile.TileContext(nc) as tc:
        with (
            tc.tile_pool(name="sb", bufs=1) as sb,
            tc.tile_pool(name="ps", bufs=4, space="PSUM") as psp,
        ):
            wq = sb.tile([128, 9, CO], dt.bfloat16)

            xs = [sb.tile([128, PACK], dt.int8, name=f"xs{g}", tag=f"xs{g}")
                  for g in range(2)]
            xq = [sb.tile([128, IMG_ELEMS], dt.bfloat16, name=f"xq{g}", tag=f"xq{g}")
                  for g in range(2)]
            os_ = [sb.tile([128, PACK], dt.float16, name=f"os{n}", tag=f"os{n}")
                   for n in range(NPC)]

            wq_flat = wq.rearrange("p t c -> p (t c)")

            # input DMA, chunked; ordered so the first chunk and then wq
            # (for PE warmup) land earliest.
            def x_dma(g, ci):
                r0, r1 = CHUNKS[ci]
                nc.sync.dma_start(
                    out=xs[g][:, r0 * W:r1 * W],
                    in_=x_in[128 * g:128 * (g + 1), r0 * W:r1 * W])

            # tap-0 weights first (32 KB): unblocks the PE warmup early
            nc.sync.dma_start(out=wq[:, 0:1, :], in_=wq_in[:, 0:1, :])
            x_dma(0, 0)
            x_dma(0, 1)
            nc.sync.dma_start(out=wq[:, 1:9, :], in_=wq_in[:, 1:9, :])
            for ci in range(2, len(CHUNKS)):
                x_dma(0, ci)
            for ci in range(len(CHUNKS)):
                x_dma(1, ci)

            # zero the padded bf16 grids.  Full-tile memsets (skinny
            # strided 16-bit border writes crash the runtime).  g=0 on
            # DVE (idle this early, fast), g=1 on the otherwise-idle
            # GpSimd.
            nc.vector.memset(xq[0][:], 0.0)
            nc.gpsimd.memset(xq[1][:], 0.0)

            # upconvert int8 -> bf16 into padded rows 1..56, chunked.
            # ACT does the relayout; the first two (tiny) chunks go on DVE
            # so the head critical path has no cross-engine hop.
            for g in range(2):
                x3 = xs[g].rearrange("p (r w) -> p r w", w=W)
                grid = xq[g][:, LEAD:LEAD + WP * WP].rearrange(
                    "p (r w) -> p r w", w=WP)
                for ci, (r0, r1) in enumerate(CHUNKS):
                    if g == 0 and ci <= 1:
                        nc.vector.tensor_scalar(
                            out=grid[:, 1 + r0:1 + r1, 1:57],
                            in0=x3[:, r0:r1, :],
                            scalar1=0.0, scalar2=None,
                            op0=mybir.AluOpType.add,
                            op1=mybir.AluOpType.bypass,
                        )
                    else:
                        nc.scalar.activation(
                            out=grid[:, 1 + r0:1 + r1, 1:57],
                            in_=x3[:, r0:r1, :],
                            func=mybir.ActivationFunctionType.Copy,
                            bias=0.0, scale=1.0,
                        )

            # PE warmup (HAM clock-gate un-throttle) overlapping the DMA
            # head.  Own psum tile + dummy DCE-guard copy whose target is
            # overwritten by the real img-0 copy later.
            if os.environ.get("KQ_WARM", "1") == "1":
                warm = psp.tile([128, 512], dt.float32, name="warm", tag="ps")
                for _ in range(N_WARM):
                    nc.tensor.matmul(
                        warm[:, 0:128], lhsT=wq[0:64, 0, :],
                        rhs=wq_flat[0:64, 0:128], start=True, stop=True,
                    )
                nc.vector.tensor_copy(os_[0][0:1, 0:1], warm[0:1, 0:1])

            for g in range(2):
                # 7 blocks of 8 output rows, processed in ITERS groups so
                # one PSUM tile spans <=2 banks; images 2g / 2g+1
                # concurrently via PE row-tiling (partition halves).
                for blocks in ITERS:
                    b0, nb = blocks[0], len(blocks)
                    ps_pair = [psp.tile([128, 1024], dt.float32,
                                        name=f"psum_g{g}b{b0}h{h}", tag="ps")
                               for h in range(2)]
                    # each 464-wide block sits bank-aligned (cols 0 and 512)
                    ps2 = [p.rearrange("p (b x) -> p b x", b=2) for p in ps_pair]
                    for t in range(9):
                        dh, dw = t // 3, t % 3
                        # h=1 (ACT-produced xq) first so PE's vector clock
                        # syncs on ACT before the h=0 matmuls, which then
                        # carry only their DVE wait (TRN2 matmul has a
                        # single sync-wait slot).
                        for h in (1, 0):
                            for bi in range(nb):
                                off = LEAD + (H0S[b0 + bi] + dh - 1) * WP + (dw - 1)
                                nc.tensor.matmul(
                                    ps2[h][:, bi, 0:BLK],
                                    lhsT=wq[64 * h:64 * (h + 1), t, :],
                                    rhs=xq[g][64 * h:64 * (h + 1), off:off + BLK],
                                    start=(t == 0), stop=(t == 8),
                                )
                    # scale + strip pad columns -> fp16;  DVE for the even
                    # image, ACT for the odd one (balance the engines).
                    # The very last group is split into row-halves so the
                    # final output DMA (and its completion receipt) is
                    # smaller.
                    last = (g == 1 and blocks is ITERS[-1])
                    row_parts = ([(0, 4), (4, 8)] if last and nb == 1
                                 else [(0, 8)])
                    for h in range(2):
                        img = 2 * g + h
                        for (q0, q1) in row_parts:
                            sel = ps2[h][:, 0:nb, 0:BLK].rearrange(
                                "p b (r w) -> p b r w", w=WP)[:, :, q0:q1, 1:57]
                            dst = os_[img].rearrange(
                                "p (b r w) -> p b r w", r=8, w=W)[
                                :, b0:b0 + nb, q0:q1]
                            if h == 0:
                                nc.vector.tensor_scalar_mul(
                                    out=dst, in0=sel, scalar1=s2)
                            else:
                                nc.scalar.activation(
                                    out=dst, in_=sel,
                                    func=mybir.ActivationFunctionType.Copy,
                                    scale=s2,
                                )
                            nc.sync.dma_start(
                                out=out[CO * img:CO * (img + 1),
                                        448 * b0 + 56 * q0:
                                        448 * (b0 + nb - 1) + 56 * q1],
                                in_=os_[img][:, 448 * b0 + 56 * q0:
                                             448 * (b0 + nb - 1) + 56 * q1],
                            )
    if not nc.is_finalized():
        nc.finalize()   # Bacc: runs wait-splitting + register allocation
    return nc


def _host_prep(x, w, alpha_x, alpha_w):
    """Quantize both operands host-side, replicating the reference's fp32
    arithmetic exactly (round-half-even of the fp32 quotient)."""
    x = np.asarray(x, dtype=np.float32)
    w = np.asarray(w, dtype=np.float32)
    ax = np.float32(max(np.float32(np.asarray(alpha_x).reshape(-1)[0]), np.float32(0)))
    aw = np.float32(max(np.float32(np.asarray(alpha_w).reshape(-1)[0]), np.float32(0)))
    step_x = np.float32(np.float32(np.float32(2.0) * ax) / np.float32(254.0))
    step_w = np.float32(np.float32(np.float32(2.0) * aw) / np.float32(254.0))
    s2 = np.float32(step_x * step_w)

    kx = np.clip(np.round((x / step_x).astype(np.float32)), -127, 127)
    kx = np.ascontiguousarray(kx.astype(np.int8))

    kw = np.clip(np.round((w / step_w).astype(np.float32)), -127, 127)
    kw = kw.reshape(CO, CI, 9).transpose(1, 2, 0)          # [ci, tap, co]
    wq = np.concatenate([kw, kw], axis=0).astype(ml_dtypes.bfloat16)
    return kx, wq, s2


def _in_maps(kx, wq):
    return [
        {
            "x": kx[NPC * c:NPC * (c + 1)].reshape(NPC * CI, PACK),
            "wq": wq,
        }
        for c in range(N_CORES)
    ]


def get_program(s2=float(np.float32(np.float32(2.0 / 254.0) ** 2))):
    key = float(np.float32(s2))
    if key not in _PROG_CACHE:
        _PROG_CACHE[key] = _build_program(key)
    return _PROG_CACHE[key]


def run_on_hw(x, w, alpha_x, alpha_w, trace=False):
    kx, wq, s2 = _host_prep(x, w, alpha_x, alpha_w)
    nc = get_program(s2)
    res = run_bass_kernel_spmd(nc, _in_maps(kx, wq),
                               list(range(N_CORES)), trace=trace)
    out = np.concatenate(
        [np.asarray(res.results[i]["out"]).reshape(NPC, CO, H, W)
         for i in range(N_CORES)], axis=0)
    return out.astype(np.float32), res


def kernel(x, w, alpha_x, alpha_w):
    out, _ = run_on_hw(x, w, alpha_x, alpha_w)
    return out


# revision 5
# speedup vs baseline: 1.0677x; 1.0677x over previous
"""Quantized 3x3 conv (8-bit symmetric STE quantization of x and w, then
stride-1 pad-1 conv) on 8 Trainium2 NeuronCores.

Strategy (v3)
-------------
Data-parallel over batch: 4 images per core (32/8).  Host pre-quantizes
both operands to integer grids (exactly the reference fp32 math):
  * x -> kx int8 in [-127,127]  (1/4 the DMA bytes of fp32)
  * w -> kw bf16 lhsT [ci, tap, co], duplicated into both partition halves
Per core:
  * kx int8 DMAs in chunked (triggers on Sync); DVE upconverts to bf16
    while relaying out to a 58-wide zero-padded grid (pad = conv padding).
  * conv = 9 shifted matmuls (K=ci=64, M=co=128) accumulating in PSUM.
    Integer products accumulate exactly in fp32 PSUM (|sum| <= 9.3e6 < 2^24).
    Two images run concurrently on the PE via row-tiling: image (2g) on
    partitions 0-63, image (2g+1) on partitions 64-127.
  * PSUM -> SBUF copy applies the final scale s2 = step_x*step_w, emits
    fp16 (rel err ~5e-4, half the output DMA bytes), strips pad columns.
    Even images on DVE (triggers via idle GpSimd — DVE can't initiate
    DMAs), odd images on ACT with its own triggers, so the ~0.6us
    trigger cost is not serialized on one sequencer (it was the whole
    output tail).
  * Host upcasts fp16 -> fp32.
"""

import os

import numpy as np
import ml_dtypes

import concourse.bass as bass
import concourse.mybir as mybir
import concourse.tile as tile
from concourse import bacc
from concourse.bass_utils import run_bass_kernel_spmd

dt = mybir.dt

N_CORES = 8
NPC = 4                # images per core
CI, CO = 64, 128
H = W = 56
WP = 58                # padded row width (56 + 2)
LEAD = 4               # guard elems before the padded grid
IMG_ELEMS = LEAD + WP * WP + 8   # 4 + 3364 + 8 = 3376
PACK = H * W           # 3136
H0S = [1 + 8 * i for i in range(7)]   # padded-row start of each 8-row block
BLK = 8 * WP           # 464 psum columns per block
N_WARM = int(os.environ.get("KQ_WARM", "8"))   # PE warmup matmuls

_PROG_CACHE = {}


def _build_program(s2):
    """One SPMD program; per-core shards differ only through in_maps.

    s2 (=step_x*step_w) is embedded as an immediate — the program is
    specialized per (alpha_x, alpha_w) value and cached."""
    s2 = float(np.float32(s2))
    nc = bacc.Bacc(None)
    x_in = nc.declare_dram_parameter("x", [NPC * CI, PACK], dt.int8, isOutput=False)
    wq_in = nc.declare_dram_parameter("wq", [128, 9, CO], dt.bfloat16, isOutput=False)
    out = nc.declare_dram_parameter("out", [NPC * CO, PACK], dt.float16, isOutput=True)

    # input chunks (data-row ranges).  g0's first chunk is tiny so block
    # 0's matmuls start as early as possible; the rest are coarse (the
    # int8 transfers are small and each trigger costs ~0.6us on Sync).
    CHUNKS = {0: [(0, 5), (5, 25), (25, 56)], 1: [(0, 28), (28, 56)]}
    ITERS = [[0], [1, 2], [3, 4], [5], [6]]
    # block group -> last data row needed (rows b0*8-1 .. b0*8+8*nb+1)
    # against chunk ends; used only for commentary, deps are via tiles.

    with tile.TileContext(nc) as tc:
        with (
            tc.tile_pool(name="sb", bufs=1) as sb,
            tc.tile_pool(name="ps", bufs=4, space="PSUM") as psp,
        ):
            wq = sb.tile([128, 9, CO], dt.bfloat16)

            xs = [sb.tile([128, PACK], dt.int8, name=f"xs{g}", tag=f"xs{g}")
                  for g in range(2)]
            xq = [sb.tile([128, IMG_ELEMS], dt.bfloat16, name=f"xq{g}", tag=f"xq{g}")
                  for g in range(2)]
            os_ = [sb.tile([128, PACK], dt.float16, name=f"os{n}", tag=f"os{n}")
                   for n in range(NPC)]

            wq_flat = wq.rearrange("p t c -> p (t c)")

            def x_dma(g, ci):
                r0, r1 = CHUNKS[g][ci]
                nc.sync.dma_start(
                    out=xs[g][:, r0 * W:r1 * W],
                    in_=x_in[128 * g:128 * (g + 1), r0 * W:r1 * W])

            # input DMA triggers, all on Sync: tap-0 weights first (32 KB,
            # unblocks PE warmup), then g0's head chunk, then the rest.
            nc.sync.dma_start(out=wq[:, 0:1, :], in_=wq_in[:, 0:1, :])
            x_dma(0, 0)
            x_dma(0, 1)
            nc.sync.dma_start(out=wq[:, 1:9, :], in_=wq_in[:, 1:9, :])
            x_dma(0, 2)
            x_dma(1, 0)
            x_dma(1, 1)

            # zero the padded bf16 grids (full-tile memsets; skinny strided
            # 16-bit border writes crash the runtime).  g=0 on DVE (fast,
            # needed first), g=1 on the otherwise-idle GpSimd.
            nc.vector.memset(xq[0][:], 0.0)
            nc.gpsimd.memset(xq[1][:], 0.0)

            # upconvert int8 -> bf16 into padded rows 1..56, all on DVE
            # (ACT's queue must stay free for PSUM-releasing output copies).
            def p2(g, ci):
                r0, r1 = CHUNKS[g][ci]
                x3 = xs[g].rearrange("p (r w) -> p r w", w=W)
                grid = xq[g][:, LEAD:LEAD + WP * WP].rearrange(
                    "p (r w) -> p r w", w=WP)
                nc.vector.tensor_scalar(
                    out=grid[:, 1 + r0:1 + r1, 1:57],
                    in0=x3[:, r0:r1, :],
                    scalar1=0.0, scalar2=None,
                    op0=mybir.AluOpType.add,
                    op1=mybir.AluOpType.bypass,
                )

            for ci in range(len(CHUNKS[0])):
                p2(0, ci)

            # PE warmup (HAM clock-gate un-throttle) overlapping the DMA
            # head.  Own psum tile + dummy DCE-guard copy (placed early in
            # the DVE queue so the warm psum buffer frees before the real
            # block groups need all 8 banks).
            warm = None
            if N_WARM:
                warm = psp.tile([128, 512], dt.float32, name="warm", tag="ps")
                for _ in range(N_WARM):
                    nc.tensor.matmul(
                        warm[:, 0:128], lhsT=wq[0:64, 0, :],
                        rhs=wq_flat[0:64, 0:128], start=True, stop=True,
                    )
                nc.vector.tensor_copy(os_[0][0:1, 0:1], warm[0:1, 0:1])

            # g1's upconvert sits after the warm-guard in DVE's queue: runs
            # in DVE's idle slot before output copies begin.
            for ci in range(len(CHUNKS[1])):
                p2(1, ci)

            for g in range(2):
                # 7 blocks of 8 output rows, processed in ITERS groups so
                # one PSUM tile spans <=2 banks; images 2g / 2g+1
                # concurrently via PE row-tiling (partition halves).
                for blocks in ITERS:
                    b0, nb = blocks[0], len(blocks)
                    ps_pair = [psp.tile([128, 1024], dt.float32,
                                        name=f"psum_g{g}b{b0}h{h}", tag="ps")
                               for h in range(2)]
                    # each 464-wide block sits bank-aligned (cols 0 and 512)
                    ps2 = [p.rearrange("p (b x) -> p b x", b=2) for p in ps_pair]
                    for t in range(9):
                        dh, dw = t // 3, t % 3
                        # h=1 first so PE's vector clock syncs before the
                        # h=0 matmuls (TRN2 matmul has one sync-wait slot).
                        for h in (1, 0):
                            for bi in range(nb):
                                off = LEAD + (H0S[b0 + bi] + dh - 1) * WP + (dw - 1)
                                nc.tensor.matmul(
                                    ps2[h][:, bi, 0:BLK],
                                    lhsT=wq[64 * h:64 * (h + 1), t, :],
                                    rhs=xq[g][64 * h:64 * (h + 1), off:off + BLK],
                                    start=(t == 0), stop=(t == 8),
                                )
                    # scale + strip pad columns -> fp16; DVE for the even
                    # image, ACT for the odd one, each engine issuing its
                    # own output-DMA trigger right after its copy.  The
                    # last group is split into row-halves so the final
                    # DMA (and its completion receipt) is smaller.
                    last = (g == 1 and blocks is ITERS[-1])
                    row_parts = ([(0, 4), (4, 8)] if last and nb == 1
                                 else [(0, 8)])
                    for h in range(2):
                        img = 2 * g + h
                        eng = nc.gpsimd if h == 0 else nc.scalar
                        for (q0, q1) in row_parts:
                            sel = ps2[h][:, 0:nb, 0:BLK].rearrange(
                                "p b (r w) -> p b r w", w=WP)[:, :, q0:q1, 1:57]
                            dst = os_[img].rearrange(
                                "p (b r w) -> p b r w", r=8, w=W)[
                                :, b0:b0 + nb, q0:q1]
                            if h == 0:
                                nc.vector.tensor_scalar_mul(
                                    out=dst, in0=sel, scalar1=s2)
                            else:
                                nc.scalar.activation(
                                    out=dst, in_=sel,
                                    func=mybir.ActivationFunctionType.Copy,
                                    scale=s2,
                                )
                            eng.dma_start(
                                out=out[CO * img:CO * (img + 1),
                                        448 * b0 + 56 * q0:
                                        448 * (b0 + nb - 1) + 56 * q1],
                                in_=os_[img][:, 448 * b0 + 56 * q0:
                                             448 * (b0 + nb - 1) + 56 * q1],
                            )
    if not nc.is_finalized():
        nc.finalize()   # Bacc: runs wait-splitting + register allocation
    return nc


def _host_prep(x, w, alpha_x, alpha_w):
    """Quantize both operands host-side, replicating the reference's fp32
    arithmetic exactly (round-half-even of the fp32 quotient)."""
    x = np.asarray(x, dtype=np.float32)
    w = np.asarray(w, dtype=np.float32)
    ax = np.float32(max(np.float32(np.asarray(alpha_x).reshape(-1)[0]), np.float32(0)))
    aw = np.float32(max(np.float32(np.asarray(alpha_w).reshape(-1)[0]), np.float32(0)))
    step_x = np.float32(np.float32(np.float32(2.0) * ax) / np.float32(254.0))
    step_w = np.float32(np.float32(np.float32(2.0) * aw) / np.float32(254.0))
    s2 = np.float32(step_x * step_w)

    kx = np.clip(np.round((x / step_x).astype(np.float32)), -127, 127)
    kx = np.ascontiguousarray(kx.astype(np.int8))

    kw = np.clip(np.round((w / step_w).astype(np.float32)), -127, 127)
    kw = kw.reshape(CO, CI, 9).transpose(1, 2, 0)          # [ci, tap, co]
    wq = np.concatenate([kw, kw], axis=0).astype(ml_dtypes.bfloat16)
    return kx, wq, s2


def _in_maps(kx, wq):
    return [
        {
            "x": kx[NPC * c:NPC * (c + 1)].reshape(NPC * CI, PACK),
            "wq": wq,
        }
        for c in range(N_CORES)
    ]


def get_program(s2=float(np.float32(np.float32(2.0 / 254.0) ** 2))):
    key = float(np.float32(s2))
    if key not in _PROG_CACHE:
        _PROG_CACHE[key] = _build_program(key)
    return _PROG_CACHE[key]


def run_on_hw(x, w, alpha_x, alpha_w, trace=False):
    kx, wq, s2 = _host_prep(x, w, alpha_x, alpha_w)
    nc = get_program(s2)
    res = run_bass_kernel_spmd(nc, _in_maps(kx, wq),
                               list(range(N_CORES)), trace=trace)
    out = np.concatenate(
        [np.asarray(res.results[i]["out"]).reshape(NPC, CO, H, W)
         for i in range(N_CORES)], axis=0)
    return out.astype(np.float32), res


def kernel(x, w, alpha_x, alpha_w):
    out, _ = run_on_hw(x, w, alpha_x, alpha_w)
    return out


# revision 7
# speedup vs baseline: 1.1248x; 1.0535x over previous
"""Quantized 3x3 conv (8-bit symmetric STE quantization of x and w, then
stride-1 pad-1 conv) on 8 Trainium2 NeuronCores.

Strategy (v4)
-------------
Data-parallel over batch: 4 images per core (32/8).  Host pre-quantizes
both operands to integer grids (exactly the reference fp32 math):
  * x -> kx int8 in [-127,127]  (1/4 the DMA bytes of fp32)
  * w -> kw bf16 lhsT [ci, tap, co], duplicated into both partition halves
Per core:
  * kx int8 DMAs in chunked (triggers on Sync); DVE upconverts to bf16
    while relaying out to a 58-wide zero-padded grid (pad = conv padding).
    g0's grid is zeroed in two pieces so block 0 only waits for the head.
  * PE warmup matmuls on a tiny self-memset tile start as soon as the
    program boots (no DMA dependency) — the PE's HAM clock-gate runs the
    first ~4us of matmuls at ~half rate, so the warm-in is spent before
    the real data lands.
  * conv = 9 shifted matmuls (K=ci=64, M=co=128) accumulating in PSUM.
    Integer products accumulate exactly in fp32 PSUM (|sum| <= 9.3e6 < 2^24).
    Two images run concurrently on the PE via row-tiling: image (2g) on
    partitions 0-63, image (2g+1) on partitions 64-127.
  * PSUM -> SBUF copy applies the final scale s2 = step_x*step_w, emits
    fp16 (rel err ~5e-4, half the output DMA bytes), strips pad columns.
    Even images on DVE with DMA triggers on Sync (idle after the input
    triggers; DVE can't initiate DMAs), odd images on ACT with its own
    triggers — the ~0.6us descriptor-gen per trigger must not serialize
    on one engine (it was the whole output tail), and GpSimd (slow 2.4us
    drain) must finish early.
  * Host upcasts fp16 -> fp32.
"""

import os

import numpy as np
import ml_dtypes

import concourse.bass as bass
import concourse.mybir as mybir
import concourse.tile as tile
from concourse import bacc
from concourse.bass_utils import run_bass_kernel_spmd

dt = mybir.dt

N_CORES = 8
NPC = 4                # images per core
CI, CO = 64, 128
H = W = 56
WP = 58                # padded row width (56 + 2)
LEAD = 4               # guard elems before the padded grid
IMG_ELEMS = LEAD + WP * WP + 8   # 4 + 3364 + 8 = 3376
PACK = H * W           # 3136
H0S = [1 + 8 * i for i in range(7)]   # padded-row start of each 8-row block
BLK = 8 * WP           # 464 psum columns per block
HEAD = LEAD + 11 * WP  # g0 grid head region: covers all of block 0's reads
N_WARM = int(os.environ.get("KQ_WARM", "12"))   # PE warmup matmuls

_PROG_CACHE = {}


def _build_program(s2):
    """One SPMD program; per-core shards differ only through in_maps.

    s2 (=step_x*step_w) is embedded as an immediate — the program is
    specialized per (alpha_x, alpha_w) value and cached."""
    s2 = float(np.float32(s2))
    nc = bacc.Bacc(None)
    x_in = nc.declare_dram_parameter("x", [NPC * CI, PACK], dt.int8, isOutput=False)
    wq_in = nc.declare_dram_parameter("wq", [128, 9, CO], dt.bfloat16, isOutput=False)
    out = nc.declare_dram_parameter("out", [NPC * CO, PACK], dt.float16, isOutput=True)

    # input chunks (data-row ranges).  g0's first two chunks are tiny so
    # block 0's matmuls start as early as possible; the rest are coarse
    # (each trigger costs ~0.6us of Sync sequencer time).
    CHUNKS = {0: [(0, 9), (9, 25), (25, 56)], 1: [(0, 28), (28, 56)]}
    ITERS = [[0], [1, 2], [3, 4], [5], [6]]

    with tile.TileContext(nc) as tc:
        with (
            tc.tile_pool(name="sb", bufs=1) as sb,
            tc.tile_pool(name="ps", bufs=4, space="PSUM") as psp,
        ):
            wq = sb.tile([128, 9, CO], dt.bfloat16)

            xs = [sb.tile([128, PACK], dt.int8, name=f"xs{g}", tag=f"xs{g}")
                  for g in range(2)]
            xq = [sb.tile([128, IMG_ELEMS], dt.bfloat16, name=f"xq{g}", tag=f"xq{g}")
                  for g in range(2)]
            os_ = [sb.tile([128, PACK], dt.float16, name=f"os{n}", tag=f"os{n}")
                   for n in range(NPC)]
            wsrc = sb.tile([64, 256], dt.bfloat16)

            def x_dma(g, ci):
                r0, r1 = CHUNKS[g][ci]
                nc.sync.dma_start(
                    out=xs[g][:, r0 * W:r1 * W],
                    in_=x_in[128 * g:128 * (g + 1), r0 * W:r1 * W])

            # input DMA triggers: x chunks on Sync, weights on ACT — the
            # two ~0.6us trigger costs and the ~1.4us descriptor-gen
            # latencies run in parallel across the two sequencers.
            x_dma(0, 0)
            nc.scalar.dma_start(out=wq[:, 0:1, :], in_=wq_in[:, 0:1, :])
            nc.scalar.dma_start(out=wq[:, 1:9, :], in_=wq_in[:, 1:9, :])
            x_dma(0, 1)
            x_dma(0, 2)
            x_dma(1, 0)
            x_dma(1, 1)

            # PE warmup (HAM clock-gate un-throttle) on a tiny self-memset
            # tile: starts right at program boot, no DMA dependency.  Own
            # psum tile + dummy DCE-guard copy.
            warm = None
            if N_WARM:
                nc.vector.memset(wsrc[:], 1.0)
                warm = psp.tile([128, 512], dt.float32, name="warm", tag="ps")
                for _ in range(N_WARM):
                    nc.tensor.matmul(
                        warm[:, 0:128], lhsT=wsrc[:, 0:128],
                        rhs=wsrc[:, 128:256], start=True, stop=True,
                    )

            # zero the padded bf16 grids (full-tile memsets; skinny strided
            # 16-bit border writes crash the runtime).  DVE zeroes only the
            # g0 head region — block 0's matmuls read just [0, HEAD) — so
            # its first upconvert is not stuck behind a 2.3us memset; the
            # otherwise-idle GpSimd zeroes the rest and all of g1.
            nc.vector.memset(xq[0][:, 0:HEAD], 0.0)
            nc.gpsimd.memset(xq[0][:, HEAD:IMG_ELEMS], 0.0)
            nc.gpsimd.memset(xq[1][:], 0.0)

            # upconvert int8 -> bf16 into padded rows 1..56, all on DVE
            # (ACT's queue must stay free for PSUM-releasing output copies).
            def p2(g, ci):
                r0, r1 = CHUNKS[g][ci]
                x3 = xs[g].rearrange("p (r w) -> p r w", w=W)
                grid = xq[g][:, LEAD:LEAD + WP * WP].rearrange(
                    "p (r w) -> p r w", w=WP)
                nc.vector.tensor_scalar(
                    out=grid[:, 1 + r0:1 + r1, 1:57],
                    in0=x3[:, r0:r1, :],
                    scalar1=0.0, scalar2=None,
                    op0=mybir.AluOpType.add,
                    op1=mybir.AluOpType.bypass,
                )

            p2(0, 0)
            p2(0, 1)
            p2(0, 2)
            if N_WARM:
                nc.vector.tensor_copy(os_[0][0:1, 0:1], warm[0:1, 0:1])
            # g1's upconvert runs in DVE's idle slot before output copies.
            p2(1, 0)
            p2(1, 1)

            for g in range(2):
                # 7 blocks of 8 output rows, processed in ITERS groups so
                # one PSUM tile spans <=2 banks; images 2g / 2g+1
                # concurrently via PE row-tiling (partition halves).
                for blocks in ITERS:
                    b0, nb = blocks[0], len(blocks)
                    ps_pair = [psp.tile([128, 1024], dt.float32,
                                        name=f"psum_g{g}b{b0}h{h}", tag="ps")
                               for h in range(2)]
                    # each 464-wide block sits bank-aligned (cols 0 and 512)
                    ps2 = [p.rearrange("p (b x) -> p b x", b=2) for p in ps_pair]
                    for t in range(9):
                        dh, dw = t // 3, t % 3
                        # h=1 first so PE's vector clock syncs before the
                        # h=0 matmuls (TRN2 matmul has one sync-wait slot).
                        for h in (1, 0):
                            for bi in range(nb):
                                off = LEAD + (H0S[b0 + bi] + dh - 1) * WP + (dw - 1)
                                nc.tensor.matmul(
                                    ps2[h][:, bi, 0:BLK],
                                    lhsT=wq[64 * h:64 * (h + 1), t, :],
                                    rhs=xq[g][64 * h:64 * (h + 1), off:off + BLK],
                                    start=(t == 0), stop=(t == 8),
                                )
                    # scale + strip pad columns -> fp16; DVE for the even
                    # image (DMA trigger via Sync), ACT for the odd one
                    # (its own trigger).
                    for h in range(2):
                        img = 2 * g + h
                        sel = ps2[h][:, 0:nb, 0:BLK].rearrange(
                            "p b (r w) -> p b r w", w=WP)[:, :, 0:8, 1:57]
                        dst = os_[img].rearrange(
                            "p (b r w) -> p b r w", r=8, w=W)[:, b0:b0 + nb]
                        if h == 0:
                            nc.vector.tensor_scalar_mul(
                                out=dst, in0=sel, scalar1=s2)
                            eng = nc.sync
                        else:
                            nc.scalar.activation(
                                out=dst, in_=sel,
                                func=mybir.ActivationFunctionType.Copy,
                                scale=s2,
                            )
                            eng = nc.scalar
                        eng.dma_start(
                            out=out[CO * img:CO * (img + 1),
                                    448 * b0:448 * (b0 + nb)],
                            in_=os_[img][:, 448 * b0:448 * (b0 + nb)],
                        )
    if not nc.is_finalized():
        nc.finalize()   # Bacc: runs wait-splitting + register allocation
    return nc


def _host_prep(x, w, alpha_x, alpha_w):
    """Quantize both operands host-side, replicating the reference's fp32
    arithmetic exactly (round-half-even of the fp32 quotient)."""
    x = np.asarray(x, dtype=np.float32)
    w = np.asarray(w, dtype=np.float32)
    ax = np.float32(max(np.float32(np.asarray(alpha_x).reshape(-1)[0]), np.float32(0)))
    aw = np.float32(max(np.float32(np.asarray(alpha_w).reshape(-1)[0]), np.float32(0)))
    step_x = np.float32(np.float32(np.float32(2.0) * ax) / np.float32(254.0))
    step_w = np.float32(np.float32(np.float32(2.0) * aw) / np.float32(254.0))
    s2 = np.float32(step_x * step_w)

    kx = np.clip(np.round((x / step_x).astype(np.float32)), -127, 127)
    kx = np.ascontiguousarray(kx.astype(np.int8))

    kw = np.clip(np.round((w / step_w).astype(np.float32)), -127, 127)
    kw = kw.reshape(CO, CI, 9).transpose(1, 2, 0)          # [ci, tap, co]
    wq = np.concatenate([kw, kw], axis=0).astype(ml_dtypes.bfloat16)
    return kx, wq, s2


def _in_maps(kx, wq):
    return [
        {
            "x": kx[NPC * c:NPC * (c + 1)].reshape(NPC * CI, PACK),
            "wq": wq,
        }
        for c in range(N_CORES)
    ]


def get_program(s2=float(np.float32(np.float32(2.0 / 254.0) ** 2))):
    key = float(np.float32(s2))
    if key not in _PROG_CACHE:
        _PROG_CACHE[key] = _build_program(key)
    return _PROG_CACHE[key]


def run_on_hw(x, w, alpha_x, alpha_w, trace=False):
    kx, wq, s2 = _host_prep(x, w, alpha_x, alpha_w)
    nc = get_program(s2)
    res = run_bass_kernel_spmd(nc, _in_maps(kx, wq),
                               list(range(N_CORES)), trace=trace)
    out = np.concatenate(
        [np.asarray(res.results[i]["out"]).reshape(NPC, CO, H, W)
         for i in range(N_CORES)], axis=0)
    return out.astype(np.float32), res


def kernel(x, w, alpha_x, alpha_w):
    out, _ = run_on_hw(x, w, alpha_x, alpha_w)
    return out
